# revision 19
# baseline (speedup 1.0000x reference)
"""Trainium2 Bass kernel for DualEncoderEpsNetwork (gnn_message_passing).

Data-parallel over B=128 graphs across 8 NeuronCores (16 graphs/core).
Feature-major layout on device: [128 features (partitions) x pairs (free)].
Matmuls run in float32r (TF32-like, full-rate fp32) with weights stationary.

Structure exploited: bond_adj is a chain, so edge_type/local form a fixed
+-3 band. The local (GIN) branch runs on a compact banded layout
(192 slots/graph vs 1024 dense). Dense-grid embedding gather collapses to
E[edge_type==0] folded into an ACT affine, with banded fixups.
softplus(x) = ln(exp(x)+1) via two ACT ops (no softplus table on TRN2).
"""

import sys

import numpy as np

sys.path.insert(0, "/opt/trn_rl_repo")

import concourse.bacc as bacc  # noqa: E402
import concourse.bass as bass  # noqa: E402
import concourse.tile as tile  # noqa: E402
from concourse import mybir  # noqa: E402
from concourse.bass_utils import run_bass_kernel_spmd  # noqa: E402

F32 = mybir.dt.float32
F32R = mybir.dt.float32r
BF16 = mybir.dt.bfloat16
AF = mybir.ActivationFunctionType
ALU = mybir.AluOpType
AX = mybir.AxisListType

# ---------------- problem constants ----------------
B, NPG, H = 128, 32, 128
CUTOFF = 10.0
NCONV_G, NCONV_L = 6, 4
NCORES = 8
G = B // NCORES            # graphs per core = 16
GN = G * NPG               # nodes per core = 512
NP_D = G * NPG * NPG       # dense pairs per core = 16384
DIAGS = [1, 2, 3, -1, -2, -3]
ND = len(DIAGS)
NP_B = ND * GN             # band slots per core = 3072, layout [d, g, i]
CHUNK = 1024               # free-dim chunk (= one graph's pair grid)
NCH = NP_D // CHUNK        # 16 chunks
LOG2 = float(np.log(2.0))

_CACHE = {}


def _np(x):
    return np.asarray(x, np.float32)


# ---------------- weight-constant packing ----------------
class WPack:
    def __init__(self):
        self.cols = []
        self.data = []

    def add(self, name, arr):
        arr = np.asarray(arr, np.float32)
        if arr.ndim == 1:
            arr = arr[:, None]
        assert arr.ndim == 2 and arr.shape[0] <= 128, (name, arr.shape)
        if arr.shape[0] < 128:
            arr = np.concatenate(
                [arr, np.zeros((128 - arr.shape[0], arr.shape[1]), np.float32)],
                0)
        off = sum(a.shape[1] for a in self.data)
        self.cols.append((name, off, arr.shape[1]))
        self.data.append(arr)
        return off

    def matrix(self):
        return np.concatenate(self.data, 1)

    def slices(self):
        return {n: (o, w) for n, o, w in self.cols}


def _prep_params(params):
    wp = WPack()   # fp32r-converted weights (matmul lhsT slices)
    bp = WPack()   # fp32 per-partition biases / scales

    def lin(p):
        return _np(p[0]), _np(p[1])

    eg, el_ = params["eenc_g"], params["eenc_l"]
    w1g, b1g = lin(eg["mlp"][0])
    w2g, b2g = lin(eg["mlp"][1])
    w1l, b1l = lin(el_["mlp"][0])
    w2l, b2l = lin(el_["mlp"][1])
    embg = _np(eg["bond_emb"])[:7]
    embl = _np(el_["bond_emb"])[:7]
    wp.add("W2g", w2g)
    wp.add("W2l", w2l)
    bp.add("b1g", b1g)
    bp.add("b1l", b1l)
    bp.add("b2g", b2g)
    bp.add("b2l", b2l)
    bp.add("E0g_s", embg[0])
    bp.add("E0g_b", embg[0] * b2g)
    for i, lay in enumerate(params["schnet"]):
        F1, c1 = lin(lay["filt"][0])
        F2, c2 = lin(lay["filt"][1])
        lin1 = _np(lay["lin1"])
        W2a, b2a = lin(lay["lin2"])
        W3a, b3a = lin(lay["lin"])
        wp.add(f"F1_{i}", F1)
        wp.add(f"F2_{i}", F2)
        wp.add(f"lin1_{i}", lin1)
        wp.add(f"W2a_{i}", W2a)
        wp.add(f"W3a_{i}", W3a)
        bp.add(f"c1_{i}", c1)
        bp.add(f"c2p_{i}", c2 - LOG2 * F2.sum(0))
        bp.add(f"b2a_{i}", b2a)
        bp.add(f"b3ap_{i}", b3a - LOG2 * W3a.sum(0))
    for i, (l1, l2) in enumerate(params["gin"]):
        G1, g1b = lin(l1)
        G2, g2b = lin(l2)
        wp.add(f"G1_{i}", G1)
        wp.add(f"G2_{i}", G2)
        bp.add(f"g1b_{i}", g1b)
        bp.add(f"g2b_{i}", g2b)
    for tag in ("g", "l"):
        (W1, b1), (W2, b2), (w3, b3) = params[f"mlp_{tag}"]
        W1, b1, W2, b2, w3 = map(_np, (W1, b1, W2, b2, w3))
        wp.add(f"H{tag}_W1a", W1[:H])
        wp.add(f"H{tag}_W1b", W1[H:])
        wp.add(f"H{tag}_W2", W2)
        wp.add(f"H{tag}_w3", w3)
        bp.add(f"H{tag}_b1", b1)
        bp.add(f"H{tag}_b2", b2)
    return wp, bp, (w1g, w1l), (embg, embl)


def _host_cond(spectrum, params, time_step):
    """Conditioning stack on host (~100 MFLOP total)."""

    def gelu_tanh(x):
        return (0.5 * x * (
            1.0 + np.tanh(np.sqrt(2.0 / np.pi).astype(np.float32)
                          * (x + 0.044715 * x**3)))).astype(np.float32)

    def mlp(x, layers, act):
        for i, (Wm, bm) in enumerate(layers):
            x = x @ _np(Wm) + _np(bm)
            if i < len(layers) - 1:
                x = act(x)
        return x

    spec_c = mlp(_np(spectrum), params["spec"], gelu_tanh)
    half = H // 2
    freqs = np.exp(np.arange(half) * (-np.log(10000.0) / (half - 1))).astype(
        np.float32)
    te = _np(time_step)[:, None] * freqs[None, :]
    te = np.concatenate([np.sin(te), np.cos(te)], -1).astype(np.float32)
    (W1, b1), (W2, b2) = params["temb"]
    tc_ = gelu_tanh(te @ _np(W1) + _np(b1)) @ _np(W2) + _np(b2)
    cond = mlp(np.concatenate([spec_c, tc_], -1).astype(np.float32),
               params["cond"], gelu_tanh)
    return cond.astype(np.float32)  # [B, H]


def _band_indices():
    out = []
    for d in DIAGS:
        if d > 0:
            out.append((d, 0, NPG - d))
        else:
            out.append((d, -d, NPG + d))
    return out


def _host_masks(pos, bond_adj):
    """Exact mirror of the reference mask math (jax ops, CPU backend —
    matches a CPU-run reference bit-for-bit)."""
    import jax
    import jax.numpy as jnp

    cpu = jax.devices("cpu")[0]
    with jax.default_device(cpu):
        return _host_masks_inner(jnp, pos, bond_adj)


def _host_masks_inner(jnp, pos, bond_adj):
    p = jnp.asarray(np.asarray(pos)).reshape(B, NPG, 3)
    badj = jnp.asarray(np.asarray(bond_adj))
    A = badj > 0
    Af = A.astype(jnp.float32)
    I = jnp.eye(NPG, dtype=bool)[None]
    A2f = Af @ Af
    hop2 = (A2f > 0) & ~A & ~I
    hop3 = ((A2f @ Af) > 0) & ~(A2f > 0) & ~A & ~I
    edge_type = jnp.where(A, badj, jnp.where(hop2, 5, jnp.where(hop3, 6, 0)))
    d = p[:, :, None, :] - p[:, None, :, :]
    dist = jnp.sqrt(jnp.sum(d * d, axis=-1) + 1e-12)
    valid = ((edge_type > 0) | (dist < CUTOFF)) & ~I
    local = edge_type > 0
    C = (dist <= CUTOFF) & valid
    return (np.asarray(edge_type, np.int32), np.asarray(dist, np.float32),
            np.asarray(valid), np.asarray(local), np.asarray(C))


# ---------------- device program ----------------
def _patch_act_tables():
    """Make every activation func this kernel uses resolve to the single
    natural_log_exp_and_others set (it contains Exp/Ln/Relu/Abs/Identity),
    so the act-table chooser never switches sets (~200 reloads avoided).
    Set order/ids are preserved; we only remove duplicate homes."""
    import concourse.hw_specs as hw_specs

    if getattr(_patch_act_tables, "done", False):
        return
    orig = hw_specs.get_activation_tables
    mine = {AF.Exp, AF.Ln, AF.Relu, AF.Abs, AF.Identity, AF.Copy}

    def dedup(arch):
        t = orig(arch)
        pref = "natural_log_exp_and_others"
        if pref in t and mine <= t[pref]:
            t = {k: (v if k == pref else (v - mine))
                 for k, v in t.items()}
        return t

    hw_specs.get_activation_tables = dedup
    bacc.get_activation_tables = dedup
    _patch_act_tables.done = True


def build_program(ws, bs, NW, NB):
    _patch_act_tables()
    nc = bacc.Bacc("TRN2", target_bir_lowering=False, debug=False,
                   num_devices=NCORES)
    binfo = _band_indices()

    el_row = nc.dram_tensor("el_row", [1, NP_D], F32, kind="ExternalInput").ap()
    el_band = nc.dram_tensor("el_band", [1, NP_B], F32,
                             kind="ExternalInput").ap()
    crow = nc.dram_tensor("crow", [1, NP_D], BF16, kind="ExternalInput").ap()
    ebg_in = nc.dram_tensor("ebg", [128, NP_B], F32, kind="ExternalInput").ap()
    ebl_in = nc.dram_tensor("ebl", [128, NP_B], F32, kind="ExternalInput").ap()
    h0_in = nc.dram_tensor("h0", [128, GN], F32, kind="ExternalInput").ap()
    hl0_in = nc.dram_tensor("hl0", [128, GN], F32, kind="ExternalInput").ap()
    cond_in = nc.dram_tensor("cond", [128, G], F32, kind="ExternalInput").ap()
    wconst = nc.dram_tensor("wconst", [128, NW], F32,
                            kind="ExternalInput").ap()
    bconst = nc.dram_tensor("bconst", [128, NB], F32,
                            kind="ExternalInput").ap()
    w1rows = nc.dram_tensor("w1rows", [1, 2 * H], F32,
                            kind="ExternalInput").ap()
    invg_out = nc.dram_tensor("invg", [1, NP_D], F32,
                              kind="ExternalOutput").ap()
    invl_out = nc.dram_tensor("invl", [1, NP_B], F32,
                              kind="ExternalOutput").ap()

    def ap3(t, col0, dims):
        return bass.AP(tensor=t.tensor, offset=t.offset + col0,
                       ap=[list(t.ap[0])] + [list(d) for d in dims])

    def mm512(ps, lhsT_list, rhs_list, n):
        """PSUM-accumulating matmul group, split into <=512-col pieces.
        lhsT_list/rhs_list: parallel lists; rhs entries are (tile, col0)."""
        for off in range(0, n, 512):
            sz = min(512, n - off)
            for k, (w, (srct, c0)) in enumerate(zip(lhsT_list, rhs_list)):
                nc.tensor.matmul(ps[:, off:off + sz], w,
                                 srct[:, c0 + off:c0 + off + sz],
                                 start=(k == 0), stop=(k == len(lhsT_list) - 1))

    with tile.TileContext(nc) as tc:
        with tc.tile_pool(name="persist", bufs=1) as persist, \
             tc.tile_pool(name="psum", bufs=3, space="PSUM") as psum, \
             tc.tile_pool(name="psc", bufs=1, space="PSUM") as psc:

            attr_g = persist.tile([128, NP_D], F32R)
            attr_l = persist.tile([128, NP_B], F32R)
            wr = persist.tile([128, NW], F32R)
            bc = persist.tile([128, NB], F32)
            w1r = persist.tile([1, 2 * H], F32)
            h = persist.tile([128, GN], F32R)
            hl = persist.tile([128, GN], F32R)
            cond_sb = persist.tile([128, G], F32)
            hg = persist.tile([128, GN], F32R)
            hlc = persist.tile([128, GN], F32R)
            zeros = persist.tile([128, 1], F32)

            def W(name):
                o, wdt = ws[name]
                return wr[:, o:o + wdt]

            def Bv(name):
                o, wdt = bs[name]
                return bc[:, o:o + wdt]

            nc.vector.memset(zeros, 0.0)
            # ---- staging: load + fp32r-convert constants ----
            with tc.tile_pool(name="stage", bufs=1) as stage:
                wtmp = stage.tile([128, NW], F32)
                nc.sync.dma_start(out=wtmp, in_=wconst)
                nc.vector.tensor_copy(wr, wtmp)
                nc.sync.dma_start(out=w1r, in_=w1rows)
                htmp = stage.tile([128, GN], F32, tag="htmp")
                nc.sync.dma_start(out=htmp, in_=h0_in)
                nc.vector.tensor_copy(h, htmp)
                hltmp = stage.tile([128, GN], F32, tag="htmp")
                nc.sync.dma_start(out=hltmp, in_=hl0_in)
                nc.vector.tensor_copy(hl, hltmp)
                nc.sync.dma_start(out=bc, in_=bconst)
                nc.sync.dma_start(out=cond_sb, in_=cond_in)

            # ---- phase A: edge encoders ----
            with tc.tile_pool(name="phA", bufs=1) as phA, \
                 tc.tile_pool(name="chA", bufs=3) as chA:
                ebg_sb = phA.tile([128, NP_B], F32)
                ebl_sb = phA.tile([128, NP_B], F32)
                elb_f = phA.tile([1, NP_B], F32)
                elb_sb = phA.tile([1, NP_B], F32R)
                nc.sync.dma_start(out=ebg_sb, in_=ebg_in)
                nc.sync.dma_start(out=ebl_sb, in_=ebl_in)
                nc.sync.dma_start(out=elb_f, in_=el_band)
                nc.vector.tensor_copy(elb_sb, elb_f)
                w1rr = phA.tile([1, 2 * H], F32R)
                nc.vector.tensor_copy(w1rr, w1r)

                def encoder_mlp(srct, c0, w1slice, b1, w2name, n):
                    ps1 = psum.tile([128, n], F32, tag="ps")
                    mm512(ps1, [w1slice], [(srct, c0)], n)
                    x1 = chA.tile([128, n], F32R, tag="x1")
                    nc.scalar.activation(out=x1, in_=ps1, func=AF.Relu,
                                         bias=b1)
                    ps2 = psum.tile([128, n], F32, tag="ps")
                    mm512(ps2, [W(w2name)], [(x1, 0)], n)
                    return ps2

                w1g_ap = w1rr[:, 0:H]
                w1l_ap = w1rr[:, H:2 * H]
                # dense global grid, one chunk per graph
                for c in range(NCH):
                    elf = chA.tile([1, CHUNK], F32, tag="elf")
                    nc.sync.dma_start(out=elf,
                                      in_=el_row[:, bass.ts(c, CHUNK)])
                    elc = chA.tile([1, CHUNK], F32R, tag="elc")
                    nc.vector.tensor_copy(elc, elf)
                    ps2 = encoder_mlp(elc, 0, w1g_ap, Bv("b1g"), "W2g", CHUNK)
                    e0b = ap3(Bv("E0g_b"), 0, [[0, CHUNK]])
                    nc.vector.scalar_tensor_tensor(
                        out=attr_g[:, bass.ts(c, CHUNK)], in0=ps2,
                        scalar=Bv("E0g_s"), in1=e0b,
                        op0=ALU.mult, op1=ALU.add)
                # band fixups (global enc) + band local encoder
                for di, (d, i0, cnt) in enumerate(binfo):
                    col0 = di * GN
                    ps2 = encoder_mlp(elb_sb, col0, w1g_ap,
                                      Bv("b1g"), "W2g", GN)
                    dense_off = i0 * (NPG + 1) + d
                    out_ap = ap3(attr_g, dense_off,
                                 [[NPG * NPG, G], [NPG + 1, cnt]])
                    in0 = bass.AP(tensor=ps2.tensor, offset=ps2.offset + i0,
                                  ap=[list(ps2.ap[0]), [NPG, G], [1, cnt]])
                    in1 = ap3(ebg_sb, col0 + i0, [[NPG, G], [1, cnt]])
                    nc.vector.scalar_tensor_tensor(
                        out=out_ap, in0=in0, scalar=Bv("b2g"), in1=in1,
                        op0=ALU.add, op1=ALU.mult)
                for di, (d, i0, cnt) in enumerate(binfo):
                    col0 = di * GN
                    ps2 = encoder_mlp(elb_sb, col0, w1l_ap,
                                      Bv("b1l"), "W2l", GN)
                    nc.vector.scalar_tensor_tensor(
                        out=attr_l[:, col0:col0 + GN], in0=ps2,
                        scalar=Bv("b2l"), in1=ebl_sb[:, col0:col0 + GN],
                        op0=ALU.add, op1=ALU.mult)

            # ---- phase B: schnet + gin ----
            with tc.tile_pool(name="phB", bufs=1) as phB, \
                 tc.tile_pool(name="chB", bufs=3) as chB, \
                 tc.tile_pool(name="chB3", bufs=3) as chB3, \
                 tc.tile_pool(name="layT", bufs=2) as layT:
                cb = phB.tile([128, NP_D], BF16)
                crow_b = bass.AP(tensor=crow.tensor, offset=crow.offset,
                                 ap=[[0, 128], [1, NP_D]])
                nc.sync.dma_start(out=cb, in_=crow_b)
                msg = phB.tile([128, NP_B], F32)

                for li in range(NCONV_G):
                    agg = layT.tile([128, GN], F32R, tag="agg")
                    x1h = layT.tile([128, GN], F32, tag="x1h")
                    psx = psum.tile([128, GN], F32, tag="ps")
                    nc.tensor.matmul(psx, W(f"lin1_{li}"), h,
                                     start=True, stop=True)
                    nc.scalar.activation(out=x1h, in_=psx, func=AF.Identity)
                    # software-pipelined: emit MM1(c+1) before MM2(c) so
                    # each engine's in-order stream never stalls the others
                    def sch_mm1(c):
                        ps_a = psum.tile([128, CHUNK], F32, tag="ps")
                        mm512(ps_a, [W(f"F1_{li}")], [(attr_g, c * CHUNK)],
                              CHUNK)
                        xhc = chB3.tile([128, CHUNK], F32, tag="xhc")
                        x1h_bc = ap3(x1h, c * NPG, [[0, NPG], [1, NPG]])
                        nc.gpsimd.tensor_tensor(out=xhc,
                                                in0=cb[:, bass.ts(c, CHUNK)],
                                                in1=x1h_bc, op=ALU.mult)
                        return ps_a, xhc

                    pend = sch_mm1(0)
                    for c in range(NCH):   # chunk c == graph c
                        ps_a, xhc = pend
                        tt = chB.tile([128, CHUNK], F32, tag="exp")
                        nc.scalar.activation(out=tt, in_=ps_a, func=AF.Exp,
                                             bias=Bv(f"c1_{li}"))
                        x1s = chB.tile([128, CHUNK], F32R, tag="x1s")
                        nc.scalar.activation(out=x1s, in_=tt, func=AF.Ln,
                                             bias=1.0)
                        if c + 1 < NCH:
                            pend = sch_mm1(c + 1)
                        ps_b = psum.tile([128, CHUNK], F32, tag="ps")
                        mm512(ps_b, [W(f"F2_{li}")], [(x1s, 0)], CHUNK)
                        p_t = chB3.tile([128, CHUNK], F32, tag="p")
                        nc.vector.scalar_tensor_tensor(
                            out=p_t, in0=ps_b, scalar=Bv(f"c2p_{li}"),
                            in1=xhc, op0=ALU.add, op1=ALU.mult)
                        p3 = bass.AP(tensor=p_t.tensor, offset=p_t.offset,
                                     ap=[list(p_t.ap[0]), [NPG, NPG],
                                         [1, NPG]])
                        with nc.allow_low_precision(
                                reason="fp32r j-sum feeds fp32r matmul"):
                            nc.vector.tensor_reduce(
                                out=agg[:, c * NPG:(c + 1) * NPG], in_=p3,
                                axis=AX.X, op=ALU.add)
                    ps1 = psum.tile([128, GN], F32, tag="ps")
                    nc.tensor.matmul(ps1, W(f"W2a_{li}"), agg,
                                     start=True, stop=True)
                    # stable softplus: relu(x) + ln(1 + exp(-|x|)); node-path
                    # inputs reach +-250, so the naive exp form overflows
                    a2 = chB.tile([128, GN], F32, tag="exp")
                    nc.scalar.activation(out=a2, in_=ps1, func=AF.Abs,
                                         bias=Bv(f"b2a_{li}"))
                    e2 = chB3.tile([128, GN], F32, tag="xhc")
                    nc.scalar.activation(out=e2, in_=a2, func=AF.Exp,
                                         scale=-1.0)
                    u2 = chB3.tile([128, GN], F32, tag="p")
                    nc.scalar.activation(out=u2, in_=e2, func=AF.Ln, bias=1.0)
                    r2n = chB.tile([128, GN], F32, tag="exp")
                    nc.scalar.activation(out=r2n, in_=ps1, func=AF.Relu,
                                         bias=Bv(f"b2a_{li}"))
                    s2 = chB.tile([128, GN], F32R, tag="x1s")
                    nc.vector.tensor_tensor(out=s2, in0=u2, in1=r2n,
                                            op=ALU.add)
                    ps2 = psum.tile([128, GN], F32, tag="ps")
                    nc.tensor.matmul(ps2, W(f"W3a_{li}"), s2,
                                     start=True, stop=True)
                    nc.vector.scalar_tensor_tensor(
                        out=h, in0=ps2, scalar=Bv(f"b3ap_{li}"), in1=h,
                        op0=ALU.add, op1=ALU.add)

                cond_bc = ap3(cond_sb, 0, [[1, G], [0, NPG]])
                nc.vector.tensor_tensor(out=hg, in0=h, in1=cond_bc,
                                        op=ALU.add)

                # GIN (banded)
                for li in range(NCONV_L):
                    nc.gpsimd.memset(msg, 0.0)
                    for di, (d, i0, cnt) in enumerate(binfo):
                        col0 = di * GN
                        out_ap = ap3(msg, col0 + i0, [[NPG, G], [1, cnt]])
                        a_ap = ap3(attr_l, col0 + i0, [[NPG, G], [1, cnt]])
                        hj_ap = ap3(hl, i0 + d, [[NPG, G], [1, cnt]])
                        nc.gpsimd.tensor_tensor(out=out_ap, in0=a_ap,
                                                in1=hj_ap, op=ALU.add)
                    nc.scalar.activation(out=msg, in_=msg, func=AF.Relu)
                    aggr = layT.tile([128, GN], F32R, tag="agg")
                    nc.vector.tensor_tensor(out=aggr, in0=hl,
                                            in1=msg[:, 0:GN], op=ALU.add)
                    for di in range(1, ND):
                        nc.vector.tensor_tensor(
                            out=aggr, in0=aggr,
                            in1=msg[:, di * GN:(di + 1) * GN], op=ALU.add)
                    ps1 = psum.tile([128, GN], F32, tag="ps")
                    nc.tensor.matmul(ps1, W(f"G1_{li}"), aggr,
                                     start=True, stop=True)
                    r1 = chB.tile([128, GN], F32R, tag="x1s")
                    nc.scalar.activation(out=r1, in_=ps1, func=AF.Relu,
                                         bias=Bv(f"g1b_{li}"))
                    ps2 = psum.tile([128, GN], F32, tag="ps")
                    nc.tensor.matmul(ps2, W(f"G2_{li}"), r1,
                                     start=True, stop=True)
                    if li < NCONV_L - 1:
                        o = chB.tile([128, GN], F32R, tag="exp")
                        nc.scalar.activation(out=o, in_=ps2, func=AF.Relu,
                                             bias=Bv(f"g2b_{li}"))
                        nc.vector.tensor_tensor(out=hl, in0=o, in1=hl,
                                                op=ALU.add)
                    else:
                        nc.vector.scalar_tensor_tensor(
                            out=hl, in0=ps2, scalar=Bv(f"g2b_{li}"), in1=hl,
                            op0=ALU.add, op1=ALU.add)
                nc.vector.tensor_tensor(out=hlc, in0=hl, in1=cond_bc,
                                        op=ALU.add)

            # ---- phase C: pair heads ----
            with tc.tile_pool(name="chC", bufs=2) as chC:

                def head_mlp(tag, prod, attr_t, acol0, out_dram_ap):
                    ps1 = psum.tile([128, CHUNK], F32, tag="ps")
                    mm512(ps1, [W(f"H{tag}_W1a"), W(f"H{tag}_W1b")],
                          [(prod, 0), (attr_t, acol0)], CHUNK)
                    r1 = chC.tile([128, CHUNK], F32R, tag="hr1")
                    nc.scalar.activation(out=r1, in_=ps1, func=AF.Relu,
                                         bias=Bv(f"H{tag}_b1"))
                    ps2 = psum.tile([64, CHUNK], F32, tag="ps")
                    mm512(ps2, [W(f"H{tag}_W2")], [(r1, 0)], CHUNK)
                    r2 = chC.tile([64, CHUNK], F32R, tag="hr2")
                    nc.scalar.activation(out=r2, in_=ps2, func=AF.Relu,
                                         bias=Bv(f"H{tag}_b2")[0:64, :])
                    ps3 = psc.tile([1, CHUNK], F32, tag="psc")
                    mm512(ps3, [W(f"H{tag}_w3")[0:64, :]], [(r2, 0)], CHUNK)
                    oc = chC.tile([1, CHUNK], F32, tag="oc")
                    nc.vector.tensor_copy(oc, ps3)
                    nc.sync.dma_start(out=out_dram_ap, in_=oc)

                def prod_g(c):
                    prod = chC.tile([128, CHUNK], F32R, tag="prod")
                    ia = ap3(hg, c * NPG, [[1, NPG], [0, NPG]])
                    ib = ap3(hg, c * NPG, [[0, NPG], [1, NPG]])
                    nc.gpsimd.tensor_tensor(out=prod, in0=ia, in1=ib,
                                            op=ALU.mult)
                    return prod

                pendp = prod_g(0)
                for c in range(NCH):   # global head, chunk c == graph c
                    prod = pendp
                    if c + 1 < NCH:
                        pendp = prod_g(c + 1)
                    head_mlp("g", prod, attr_g, c * CHUNK,
                             invg_out[:, bass.ts(c, CHUNK)])

                for c in range(NP_B // CHUNK):   # local head: 2 diagonals/chunk
                    prod = chC.tile([128, CHUNK], F32R, tag="prod")
                    nc.vector.tensor_copy(prod, ap3(zeros, 0, [[0, CHUNK]]))
                    for di in (2 * c, 2 * c + 1):
                        d, i0, cnt = binfo[di]
                        col0 = di * GN - c * CHUNK
                        ia = ap3(hlc, i0, [[NPG, G], [1, cnt]])
                        ib = ap3(hlc, i0 + d, [[NPG, G], [1, cnt]])
                        out_ap = ap3(prod, col0 + i0, [[NPG, G], [1, cnt]])
                        nc.vector.tensor_tensor(out=out_ap, in0=ia, in1=ib,
                                                op=ALU.mult)
                    head_mlp("l", prod, attr_l, c * CHUNK,
                             invl_out[:, bass.ts(c, CHUNK)])

    nc.compile()
    return nc


# ---------------- host orchestration ----------------
def _prep(inputs):
    import ml_dtypes

    pos = _np(inputs["pos"])
    spectrum = inputs["spectrum"]
    params = inputs["params"]
    atom_type = np.asarray(inputs["atom_type"], np.int64)
    time_step = np.asarray(inputs["time_step"], np.int64)
    bond_adj = np.asarray(inputs["bond_adj"], np.int32)

    edge_type, dist, valid, local, C = _host_masks(pos, bond_adj)
    wp, bp, (w1g, w1l), (embg, embl) = _prep_params(params)
    cond = _host_cond(spectrum, params, time_step)

    schnet_emb = _np(params["schnet_emb"])
    gin_emb = _np(params["gin_emb"])
    z = atom_type.reshape(B, NPG)
    h0 = schnet_emb[z]
    hl0 = gin_emb[z]

    binfo = _band_indices()
    wmat = wp.matrix()
    bmat = bp.matrix()
    w1row = np.concatenate([w1g.reshape(1, H), w1l.reshape(1, H)], 1)
    in_maps = []
    for core in range(NCORES):
        gs = slice(core * G, (core + 1) * G)
        d_c = dist[gs]
        et_c = edge_type[gs]
        C_c = C[gs].astype(np.float32)

        el_band = np.zeros((ND, G, NPG), np.float32)
        ebg_t = np.zeros((ND, G, NPG, H), np.float32)
        ebl_t = np.zeros((ND, G, NPG, H), np.float32)
        for di, (d, i0, cnt) in enumerate(binfo):
            ii = np.arange(i0, i0 + cnt)
            jj = ii + d
            el_band[di][:, ii] = d_c[:, ii, jj]
            et_d = et_c[:, ii, jj]
            ebg_t[di][:, ii] = embg[et_d]
            ebl_t[di][:, ii] = embl[et_d]

        m = {
            "el_row": np.ascontiguousarray(d_c.reshape(1, NP_D)),
            "el_band": np.ascontiguousarray(el_band.reshape(1, NP_B)),
            "crow": C_c.reshape(1, NP_D).astype(ml_dtypes.bfloat16),
            "ebg": np.ascontiguousarray(ebg_t.reshape(NP_B, H).T),
            "ebl": np.ascontiguousarray(ebl_t.reshape(NP_B, H).T),
            "h0": np.ascontiguousarray(h0[gs].reshape(GN, H).T),
            "hl0": np.ascontiguousarray(hl0[gs].reshape(GN, H).T),
            "cond": np.ascontiguousarray(cond[gs].T),
            "wconst": wmat,
            "bconst": bmat,
            "w1rows": w1row,
        }
        in_maps.append(m)

    hostdata = dict(edge_type=edge_type, dist=dist, valid=valid, local=local,
                    params=params)
    return in_maps, hostdata


def _postprocess(results, hostdata):
    params = hostdata["params"]
    b3g = float(np.asarray(params["mlp_g"][2][1]).reshape(-1)[0])
    b3l = float(np.asarray(params["mlp_l"][2][1]).reshape(-1)[0])
    vf = hostdata["valid"].astype(np.float32)
    lm = hostdata["local"].astype(np.float32)

    inv_g = np.zeros((B, NPG, NPG), np.float32)
    inv_l = np.zeros((B, NPG, NPG), np.float32)
    binfo = _band_indices()
    garr = np.arange(G)
    for core, r in enumerate(results):
        g0 = core * G
        inv_g[g0:g0 + G] = r["invg"].reshape(G, NPG, NPG) + b3g
        band = r["invl"].reshape(ND, G, NPG) + b3l
        for di, (d, i0, cnt) in enumerate(binfo):
            ii = np.arange(i0, i0 + cnt)
            inv_l[(g0 + garr)[:, None], ii[None, :], (ii + d)[None, :]] = \
                band[di][:, ii]
    inv_g = np.where(vf > 0, inv_g, 0.0).astype(np.float32) * vf
    inv_l = np.where(lm > 0, inv_l, 0.0).astype(np.float32) * lm
    return inv_g[..., None], inv_l[..., None]


def _numpy_fallback(inputs, edge_type, dist, valid, local, C):
    """Reference-faithful numpy path, used only if bond_adj is not the
    chain topology the banded device layout assumes."""
    params = inputs["params"]

    def lin(p):
        return _np(p[0]), _np(p[1])

    def sp(x):
        return np.logaddexp(0.0, x).astype(np.float32)

    relu = lambda x: np.maximum(x, 0)
    el = dist[..., None]

    def edge_attr(ep):
        (W1, b1), (W2, b2) = ep["mlp"]
        x1 = relu(el * _np(W1)[None, None, None, 0] + _np(b1))
        return (x1 @ _np(W2) + _np(b2)) * _np(ep["bond_emb"])[edge_type]

    attr_g = edge_attr(params["eenc_g"])
    attr_l = edge_attr(params["eenc_l"])
    cond = _host_cond(inputs["spectrum"], params, inputs["time_step"])
    z = np.asarray(inputs["atom_type"], np.int64).reshape(B, NPG)
    h = _np(params["schnet_emb"])[z]
    Cf = C.astype(np.float32)
    LG2 = np.float32(LOG2)
    for lay in params["schnet"]:
        F1, c1 = lin(lay["filt"][0])
        F2, c2 = lin(lay["filt"][1])
        Wf = ((sp(attr_g @ F1 + c1) - LG2) @ F2 + c2) * Cf[..., None]
        x1 = h @ _np(lay["lin1"])
        agg = np.einsum("bjf,bijf->bif", x1, Wf)
        W2a, b2a = lin(lay["lin2"])
        W3a, b3a = lin(lay["lin"])
        h = h + (sp(agg @ W2a + b2a) - LG2) @ W3a + b3a
    hg = h + cond[:, None, :]
    lm = local.astype(np.float32)
    hl = _np(params["gin_emb"])[z]
    for i, (l1, l2) in enumerate(params["gin"]):
        msg = relu(hl[:, None, :, :] + attr_l) * lm[..., None]
        G1, g1b = lin(l1)
        G2, g2b = lin(l2)
        out = relu((hl + msg.sum(axis=2)) @ G1 + g1b) @ G2 + g2b
        if i < NCONV_L - 1:
            out = relu(out)
        hl = out + hl
    hl = hl + cond[:, None, :]

    def head(hx, attr, mp, mask):
        (W1, b1), (W2, b2), (w3, b3) = [lin(p) for p in mp]
        hp = np.concatenate([hx[:, :, None, :] * hx[:, None, :, :], attr], -1)
        return (relu(relu(hp @ W1 + b1) @ W2 + b2) @ w3 + b3) * mask

    vf = valid.astype(np.float32)[..., None]
    inv_g = head(hg, attr_g, params["mlp_g"], vf)
    inv_l = head(hl, attr_l, params["mlp_l"], lm[..., None])
    return inv_g.astype(np.float32), inv_l.astype(np.float32)


def _is_chain(local):
    band = np.zeros((NPG, NPG), bool)
    for d in (1, 2, 3):
        band |= np.eye(NPG, k=d, dtype=bool) | np.eye(NPG, k=-d, dtype=bool)
    return np.array_equal(local, np.broadcast_to(band, local.shape))


def kernel(pos, spectrum, params, atom_type, time_step, bond_adj):
    in_maps, hostdata = _prep(dict(pos=pos, spectrum=spectrum, params=params,
                                   atom_type=atom_type, time_step=time_step,
                                   bond_adj=bond_adj))
    if not _is_chain(hostdata["local"]):
        inputs = dict(pos=pos, spectrum=spectrum, params=params,
                      atom_type=atom_type, time_step=time_step,
                      bond_adj=bond_adj)
        _, dist, valid, local = (hostdata["edge_type"], hostdata["dist"],
                                 hostdata["valid"], hostdata["local"])
        inv_g, inv_l = _numpy_fallback(inputs, hostdata["edge_type"], dist,
                                       valid, local,
                                       _host_masks(pos, bond_adj)[4])
        el = hostdata["dist"][..., None]
        return (inv_g, inv_l, hostdata["edge_type"], el, hostdata["valid"],
                hostdata["local"])
    if "nc" not in _CACHE:
        wp, bp, _, _ = _prep_params(params)
        _CACHE["nc"] = build_program(wp.slices(), bp.slices(),
                                     wp.matrix().shape[1],
                                     bp.matrix().shape[1])
    res = run_bass_kernel_spmd(_CACHE["nc"], in_maps,
                               core_ids=list(range(NCORES)))
    inv_g, inv_l = _postprocess(res.results, hostdata)
    el = hostdata["dist"][..., None]
    return (inv_g, inv_l, hostdata["edge_type"], el, hostdata["valid"],
            hostdata["local"])


# revision 21
# speedup vs baseline: 1.0266x; 1.0266x over previous
"""Trainium2 Bass kernel for DualEncoderEpsNetwork (gnn_message_passing).

Data-parallel over B=128 graphs across 8 NeuronCores (16 graphs/core).
Feature-major layout on device: [128 features (partitions) x pairs (free)].
Matmuls run in float32r (TF32-like, full-rate fp32) with weights stationary.

Structure exploited: bond_adj is a chain, so edge_type/local form a fixed
+-3 band. The local (GIN) branch runs on a compact banded layout
(192 slots/graph vs 1024 dense). Dense-grid embedding gather collapses to
E[edge_type==0] folded into an ACT affine, with banded fixups.
softplus(x) = ln(exp(x)+1) via two ACT ops (no softplus table on TRN2).
"""

import sys

import numpy as np

sys.path.insert(0, "/opt/trn_rl_repo")

import concourse.bacc as bacc  # noqa: E402
import concourse.bass as bass  # noqa: E402
import concourse.tile as tile  # noqa: E402
from concourse import mybir  # noqa: E402
from concourse.bass_utils import run_bass_kernel_spmd  # noqa: E402

F32 = mybir.dt.float32
F32R = mybir.dt.float32r
BF16 = mybir.dt.bfloat16
AF = mybir.ActivationFunctionType
ALU = mybir.AluOpType
AX = mybir.AxisListType

# ---------------- problem constants ----------------
B, NPG, H = 128, 32, 128
CUTOFF = 10.0
NCONV_G, NCONV_L = 6, 4
NCORES = 8
G = B // NCORES            # graphs per core = 16
GN = G * NPG               # nodes per core = 512
NP_D = G * NPG * NPG       # dense pairs per core = 16384
DIAGS = [1, 2, 3, -1, -2, -3]
ND = len(DIAGS)
NP_B = ND * GN             # band slots per core = 3072, layout [d, g, i]
CHUNK = 1024               # free-dim chunk (= one graph's pair grid)
NCH = NP_D // CHUNK        # 16 chunks
LOG2 = float(np.log(2.0))

_CACHE = {}


def _np(x):
    return np.asarray(x, np.float32)


# ---------------- weight-constant packing ----------------
class WPack:
    def __init__(self):
        self.cols = []
        self.data = []

    def add(self, name, arr):
        arr = np.asarray(arr, np.float32)
        if arr.ndim == 1:
            arr = arr[:, None]
        assert arr.ndim == 2 and arr.shape[0] <= 128, (name, arr.shape)
        if arr.shape[0] < 128:
            arr = np.concatenate(
                [arr, np.zeros((128 - arr.shape[0], arr.shape[1]), np.float32)],
                0)
        off = sum(a.shape[1] for a in self.data)
        self.cols.append((name, off, arr.shape[1]))
        self.data.append(arr)
        return off

    def matrix(self):
        return np.concatenate(self.data, 1)

    def slices(self):
        return {n: (o, w) for n, o, w in self.cols}


def _prep_params(params):
    wp = WPack()   # fp32r-converted weights (matmul lhsT slices)
    bp = WPack()   # fp32 per-partition biases / scales

    def lin(p):
        return _np(p[0]), _np(p[1])

    eg, el_ = params["eenc_g"], params["eenc_l"]
    w1g, b1g = lin(eg["mlp"][0])
    w2g, b2g = lin(eg["mlp"][1])
    w1l, b1l = lin(el_["mlp"][0])
    w2l, b2l = lin(el_["mlp"][1])
    embg = _np(eg["bond_emb"])[:7]
    embl = _np(el_["bond_emb"])[:7]
    wp.add("W2g", w2g)
    wp.add("W2l", w2l)
    bp.add("b1g", b1g)
    bp.add("b1l", b1l)
    bp.add("b2g", b2g)
    bp.add("b2l", b2l)
    bp.add("E0g_s", embg[0])
    bp.add("E0g_b", embg[0] * b2g)
    for i, lay in enumerate(params["schnet"]):
        F1, c1 = lin(lay["filt"][0])
        F2, c2 = lin(lay["filt"][1])
        lin1 = _np(lay["lin1"])
        W2a, b2a = lin(lay["lin2"])
        W3a, b3a = lin(lay["lin"])
        wp.add(f"F1_{i}", F1)
        wp.add(f"F2_{i}", F2)
        wp.add(f"lin1_{i}", lin1)
        wp.add(f"W2a_{i}", W2a)
        wp.add(f"W3a_{i}", W3a)
        bp.add(f"c1_{i}", c1)
        bp.add(f"c2p_{i}", c2 - LOG2 * F2.sum(0))
        bp.add(f"b2a_{i}", b2a)
        bp.add(f"b3ap_{i}", b3a - LOG2 * W3a.sum(0))
    for i, (l1, l2) in enumerate(params["gin"]):
        G1, g1b = lin(l1)
        G2, g2b = lin(l2)
        wp.add(f"G1_{i}", G1)
        wp.add(f"G2_{i}", G2)
        bp.add(f"g1b_{i}", g1b)
        bp.add(f"g2b_{i}", g2b)
    for tag in ("g", "l"):
        (W1, b1), (W2, b2), (w3, b3) = params[f"mlp_{tag}"]
        W1, b1, W2, b2, w3 = map(_np, (W1, b1, W2, b2, w3))
        wp.add(f"H{tag}_W1a", W1[:H])
        wp.add(f"H{tag}_W1b", W1[H:])
        wp.add(f"H{tag}_W2", W2)
        wp.add(f"H{tag}_w3", w3)
        bp.add(f"H{tag}_b1", b1)
        bp.add(f"H{tag}_b2", b2)
    return wp, bp, (w1g, w1l), (embg, embl)


def _host_cond(spectrum, params, time_step):
    """Conditioning stack on host (~100 MFLOP total)."""

    def gelu_tanh(x):
        return (0.5 * x * (
            1.0 + np.tanh(np.sqrt(2.0 / np.pi).astype(np.float32)
                          * (x + 0.044715 * x**3)))).astype(np.float32)

    def mlp(x, layers, act):
        for i, (Wm, bm) in enumerate(layers):
            x = x @ _np(Wm) + _np(bm)
            if i < len(layers) - 1:
                x = act(x)
        return x

    spec_c = mlp(_np(spectrum), params["spec"], gelu_tanh)
    half = H // 2
    freqs = np.exp(np.arange(half) * (-np.log(10000.0) / (half - 1))).astype(
        np.float32)
    te = _np(time_step)[:, None] * freqs[None, :]
    te = np.concatenate([np.sin(te), np.cos(te)], -1).astype(np.float32)
    (W1, b1), (W2, b2) = params["temb"]
    tc_ = gelu_tanh(te @ _np(W1) + _np(b1)) @ _np(W2) + _np(b2)
    cond = mlp(np.concatenate([spec_c, tc_], -1).astype(np.float32),
               params["cond"], gelu_tanh)
    return cond.astype(np.float32)  # [B, H]


def _band_indices():
    out = []
    for d in DIAGS:
        if d > 0:
            out.append((d, 0, NPG - d))
        else:
            out.append((d, -d, NPG + d))
    return out


def _host_masks(pos, bond_adj):
    """Exact mirror of the reference mask math (jax ops, CPU backend —
    matches a CPU-run reference bit-for-bit)."""
    import jax
    import jax.numpy as jnp

    cpu = jax.devices("cpu")[0]
    with jax.default_device(cpu):
        return _host_masks_inner(jnp, pos, bond_adj)


def _host_masks_inner(jnp, pos, bond_adj):
    p = jnp.asarray(np.asarray(pos)).reshape(B, NPG, 3)
    badj = jnp.asarray(np.asarray(bond_adj))
    A = badj > 0
    Af = A.astype(jnp.float32)
    I = jnp.eye(NPG, dtype=bool)[None]
    A2f = Af @ Af
    hop2 = (A2f > 0) & ~A & ~I
    hop3 = ((A2f @ Af) > 0) & ~(A2f > 0) & ~A & ~I
    edge_type = jnp.where(A, badj, jnp.where(hop2, 5, jnp.where(hop3, 6, 0)))
    d = p[:, :, None, :] - p[:, None, :, :]
    dist = jnp.sqrt(jnp.sum(d * d, axis=-1) + 1e-12)
    valid = ((edge_type > 0) | (dist < CUTOFF)) & ~I
    local = edge_type > 0
    C = (dist <= CUTOFF) & valid
    return (np.asarray(edge_type, np.int32), np.asarray(dist, np.float32),
            np.asarray(valid), np.asarray(local), np.asarray(C))


# ---------------- device program ----------------
def _patch_act_tables():
    """Make every activation func this kernel uses resolve to the single
    natural_log_exp_and_others set (it contains Exp/Ln/Relu/Abs/Identity),
    so the act-table chooser never switches sets (~200 reloads avoided).
    Set order/ids are preserved; we only remove duplicate homes."""
    import concourse.hw_specs as hw_specs

    if getattr(_patch_act_tables, "done", False):
        return
    orig = hw_specs.get_activation_tables
    mine = {AF.Exp, AF.Ln, AF.Relu, AF.Abs, AF.Identity, AF.Copy}

    def dedup(arch):
        t = orig(arch)
        pref = "natural_log_exp_and_others"
        if pref in t and mine <= t[pref]:
            t = {k: (v if k == pref else (v - mine))
                 for k, v in t.items()}
        return t

    hw_specs.get_activation_tables = dedup
    bacc.get_activation_tables = dedup
    _patch_act_tables.done = True


def build_program(ws, bs, NW, NB):
    _patch_act_tables()
    nc = bacc.Bacc("TRN2", target_bir_lowering=False, debug=False,
                   num_devices=NCORES)
    binfo = _band_indices()

    el_row = nc.dram_tensor("el_row", [1, NP_D], F32, kind="ExternalInput").ap()
    el_band = nc.dram_tensor("el_band", [1, NP_B], F32,
                             kind="ExternalInput").ap()
    crow = nc.dram_tensor("crow", [1, NP_D], BF16, kind="ExternalInput").ap()
    ebg_in = nc.dram_tensor("ebg", [128, NP_B], F32, kind="ExternalInput").ap()
    ebl_in = nc.dram_tensor("ebl", [128, NP_B], F32, kind="ExternalInput").ap()
    h0_in = nc.dram_tensor("h0", [128, GN], F32, kind="ExternalInput").ap()
    hl0_in = nc.dram_tensor("hl0", [128, GN], F32, kind="ExternalInput").ap()
    cond_in = nc.dram_tensor("cond", [128, G], F32, kind="ExternalInput").ap()
    wconst = nc.dram_tensor("wconst", [128, NW], F32,
                            kind="ExternalInput").ap()
    bconst = nc.dram_tensor("bconst", [128, NB], F32,
                            kind="ExternalInput").ap()
    w1rows = nc.dram_tensor("w1rows", [1, 2 * H], F32,
                            kind="ExternalInput").ap()
    invg_out = nc.dram_tensor("invg", [1, NP_D], F32,
                              kind="ExternalOutput").ap()
    invl_out = nc.dram_tensor("invl", [1, NP_B], F32,
                              kind="ExternalOutput").ap()

    def ap3(t, col0, dims):
        return bass.AP(tensor=t.tensor, offset=t.offset + col0,
                       ap=[list(t.ap[0])] + [list(d) for d in dims])

    def mm512(ps, lhsT_list, rhs_list, n):
        """PSUM-accumulating matmul group, split into <=512-col pieces.
        lhsT_list/rhs_list: parallel lists; rhs entries are (tile, col0)."""
        for off in range(0, n, 512):
            sz = min(512, n - off)
            for k, (w, (srct, c0)) in enumerate(zip(lhsT_list, rhs_list)):
                nc.tensor.matmul(ps[:, off:off + sz], w,
                                 srct[:, c0 + off:c0 + off + sz],
                                 start=(k == 0), stop=(k == len(lhsT_list) - 1))

    with tile.TileContext(nc) as tc:
        with tc.tile_pool(name="persist", bufs=1) as persist, \
             tc.tile_pool(name="psum", bufs=3, space="PSUM") as psum, \
             tc.tile_pool(name="psc", bufs=1, space="PSUM") as psc:

            attr_g = persist.tile([128, NP_D], F32R)
            attr_l = persist.tile([128, NP_B], F32R)
            wr = persist.tile([128, NW], F32R)
            bc = persist.tile([128, NB], F32)
            w1r = persist.tile([1, 2 * H], F32)
            h = persist.tile([128, GN], F32R)
            hl = persist.tile([128, GN], F32R)
            cond_sb = persist.tile([128, G], F32)
            hg = persist.tile([128, GN], F32R)
            hlc = persist.tile([128, GN], F32R)
            zeros = persist.tile([128, 1], F32)

            def W(name):
                o, wdt = ws[name]
                return wr[:, o:o + wdt]

            def Bv(name):
                o, wdt = bs[name]
                return bc[:, o:o + wdt]

            nc.vector.memset(zeros, 0.0)
            # ---- staging: load + fp32r-convert constants ----
            with tc.tile_pool(name="stage", bufs=1) as stage:
                wtmp = stage.tile([128, NW], F32)
                nc.sync.dma_start(out=wtmp, in_=wconst)
                nc.vector.tensor_copy(wr, wtmp)
                nc.sync.dma_start(out=w1r, in_=w1rows)
                htmp = stage.tile([128, GN], F32, tag="htmp")
                nc.sync.dma_start(out=htmp, in_=h0_in)
                nc.vector.tensor_copy(h, htmp)
                hltmp = stage.tile([128, GN], F32, tag="htmp")
                nc.sync.dma_start(out=hltmp, in_=hl0_in)
                nc.vector.tensor_copy(hl, hltmp)
                nc.sync.dma_start(out=bc, in_=bconst)
                nc.sync.dma_start(out=cond_sb, in_=cond_in)

            # ---- phase A: edge encoders ----
            with tc.tile_pool(name="phA", bufs=1) as phA, \
                 tc.tile_pool(name="chA", bufs=3) as chA:
                ebg_sb = phA.tile([128, NP_B], F32)
                ebl_sb = phA.tile([128, NP_B], F32)
                elb_f = phA.tile([1, NP_B], F32)
                elb_sb = phA.tile([1, NP_B], F32R)
                nc.sync.dma_start(out=ebg_sb, in_=ebg_in)
                nc.sync.dma_start(out=ebl_sb, in_=ebl_in)
                nc.sync.dma_start(out=elb_f, in_=el_band)
                nc.vector.tensor_copy(elb_sb, elb_f)
                w1rr = phA.tile([1, 2 * H], F32R)
                nc.vector.tensor_copy(w1rr, w1r)

                def encoder_mlp(srct, c0, w1slice, b1, w2name, n):
                    ps1 = psum.tile([128, n], F32, tag="ps")
                    mm512(ps1, [w1slice], [(srct, c0)], n)
                    x1 = chA.tile([128, n], F32R, tag="x1")
                    nc.scalar.activation(out=x1, in_=ps1, func=AF.Relu,
                                         bias=b1)
                    ps2 = psum.tile([128, n], F32, tag="ps")
                    mm512(ps2, [W(w2name)], [(x1, 0)], n)
                    return ps2

                w1g_ap = w1rr[:, 0:H]
                w1l_ap = w1rr[:, H:2 * H]
                # dense global grid, one chunk per graph
                for c in range(NCH):
                    elf = chA.tile([1, CHUNK], F32, tag="elf")
                    nc.sync.dma_start(out=elf,
                                      in_=el_row[:, bass.ts(c, CHUNK)])
                    elc = chA.tile([1, CHUNK], F32R, tag="elc")
                    nc.vector.tensor_copy(elc, elf)
                    ps2 = encoder_mlp(elc, 0, w1g_ap, Bv("b1g"), "W2g", CHUNK)
                    e0b = ap3(Bv("E0g_b"), 0, [[0, CHUNK]])
                    nc.vector.scalar_tensor_tensor(
                        out=attr_g[:, bass.ts(c, CHUNK)], in0=ps2,
                        scalar=Bv("E0g_s"), in1=e0b,
                        op0=ALU.mult, op1=ALU.add)
                # band fixups (global enc) + band local encoder
                for di, (d, i0, cnt) in enumerate(binfo):
                    col0 = di * GN
                    ps2 = encoder_mlp(elb_sb, col0, w1g_ap,
                                      Bv("b1g"), "W2g", GN)
                    dense_off = i0 * (NPG + 1) + d
                    out_ap = ap3(attr_g, dense_off,
                                 [[NPG * NPG, G], [NPG + 1, cnt]])
                    in0 = bass.AP(tensor=ps2.tensor, offset=ps2.offset + i0,
                                  ap=[list(ps2.ap[0]), [NPG, G], [1, cnt]])
                    in1 = ap3(ebg_sb, col0 + i0, [[NPG, G], [1, cnt]])
                    nc.vector.scalar_tensor_tensor(
                        out=out_ap, in0=in0, scalar=Bv("b2g"), in1=in1,
                        op0=ALU.add, op1=ALU.mult)
                for di, (d, i0, cnt) in enumerate(binfo):
                    col0 = di * GN
                    ps2 = encoder_mlp(elb_sb, col0, w1l_ap,
                                      Bv("b1l"), "W2l", GN)
                    nc.vector.scalar_tensor_tensor(
                        out=attr_l[:, col0:col0 + GN], in0=ps2,
                        scalar=Bv("b2l"), in1=ebl_sb[:, col0:col0 + GN],
                        op0=ALU.add, op1=ALU.mult)

            # ---- phase B: schnet + gin ----
            with tc.tile_pool(name="phB", bufs=1) as phB, \
                 tc.tile_pool(name="chB", bufs=2) as chB, \
                 tc.tile_pool(name="chB3", bufs=3) as chB3, \
                 tc.tile_pool(name="layT", bufs=2) as layT:
                cb = phB.tile([128, NP_D], BF16)
                crow_b = bass.AP(tensor=crow.tensor, offset=crow.offset,
                                 ap=[[0, 128], [1, NP_D]])
                nc.sync.dma_start(out=cb, in_=crow_b)
                msg = phB.tile([128, NP_B], F32)

                # GIN (banded) — emitted interleaved with schnet layers
                # to fill their pipeline bubbles (independent branches)
                def gin_layer(li):
                    nc.gpsimd.memset(msg, 0.0)
                    for di, (d, i0, cnt) in enumerate(binfo):
                        col0 = di * GN
                        out_ap = ap3(msg, col0 + i0, [[NPG, G], [1, cnt]])
                        a_ap = ap3(attr_l, col0 + i0, [[NPG, G], [1, cnt]])
                        hj_ap = ap3(hl, i0 + d, [[NPG, G], [1, cnt]])
                        nc.gpsimd.tensor_tensor(out=out_ap, in0=a_ap,
                                                in1=hj_ap, op=ALU.add)
                    nc.scalar.activation(out=msg, in_=msg, func=AF.Relu)
                    aggr = layT.tile([128, GN], F32R, tag="aggr")
                    nc.vector.tensor_tensor(out=aggr, in0=hl,
                                            in1=msg[:, 0:GN], op=ALU.add)
                    for di in range(1, ND):
                        nc.vector.tensor_tensor(
                            out=aggr, in0=aggr,
                            in1=msg[:, di * GN:(di + 1) * GN], op=ALU.add)
                    ps1 = psum.tile([128, GN], F32, tag="ps")
                    nc.tensor.matmul(ps1, W(f"G1_{li}"), aggr,
                                     start=True, stop=True)
                    r1 = chB.tile([128, GN], F32R, tag="x1s")
                    nc.scalar.activation(out=r1, in_=ps1, func=AF.Relu,
                                         bias=Bv(f"g1b_{li}"))
                    ps2 = psum.tile([128, GN], F32, tag="ps")
                    nc.tensor.matmul(ps2, W(f"G2_{li}"), r1,
                                     start=True, stop=True)
                    if li < NCONV_L - 1:
                        o = chB.tile([128, GN], F32R, tag="exp")
                        nc.scalar.activation(out=o, in_=ps2, func=AF.Relu,
                                             bias=Bv(f"g2b_{li}"))
                        nc.vector.tensor_tensor(out=hl, in0=o, in1=hl,
                                                op=ALU.add)
                    else:
                        nc.vector.scalar_tensor_tensor(
                            out=hl, in0=ps2, scalar=Bv(f"g2b_{li}"), in1=hl,
                            op0=ALU.add, op1=ALU.add)

                for li in range(NCONV_G):
                    agg = layT.tile([128, GN], F32R, tag="agg")
                    x1h = layT.tile([128, GN], F32, tag="x1h")
                    psx = psum.tile([128, GN], F32, tag="ps")
                    nc.tensor.matmul(psx, W(f"lin1_{li}"), h,
                                     start=True, stop=True)
                    nc.scalar.activation(out=x1h, in_=psx, func=AF.Identity)
                    # software-pipelined: emit MM1(c+1) before MM2(c) so
                    # each engine's in-order stream never stalls the others
                    def sch_mm1(c):
                        ps_a = psum.tile([128, CHUNK], F32, tag="ps")
                        mm512(ps_a, [W(f"F1_{li}")], [(attr_g, c * CHUNK)],
                              CHUNK)
                        xhc = chB3.tile([128, CHUNK], F32, tag="xhc")
                        x1h_bc = ap3(x1h, c * NPG, [[0, NPG], [1, NPG]])
                        nc.gpsimd.tensor_tensor(out=xhc,
                                                in0=cb[:, bass.ts(c, CHUNK)],
                                                in1=x1h_bc, op=ALU.mult)
                        return ps_a, xhc

                    pend = sch_mm1(0)
                    for c in range(NCH):   # chunk c == graph c
                        ps_a, xhc = pend
                        tt = chB.tile([128, CHUNK], F32, tag="exp")
                        nc.scalar.activation(out=tt, in_=ps_a, func=AF.Exp,
                                             bias=Bv(f"c1_{li}"))
                        x1s = chB.tile([128, CHUNK], F32R, tag="x1s")
                        nc.scalar.activation(out=x1s, in_=tt, func=AF.Ln,
                                             bias=1.0)
                        if c + 1 < NCH:
                            pend = sch_mm1(c + 1)
                        ps_b = psum.tile([128, CHUNK], F32, tag="ps")
                        mm512(ps_b, [W(f"F2_{li}")], [(x1s, 0)], CHUNK)
                        p_t = chB3.tile([128, CHUNK], F32, tag="p")
                        nc.vector.scalar_tensor_tensor(
                            out=p_t, in0=ps_b, scalar=Bv(f"c2p_{li}"),
                            in1=xhc, op0=ALU.add, op1=ALU.mult)
                        p3 = bass.AP(tensor=p_t.tensor, offset=p_t.offset,
                                     ap=[list(p_t.ap[0]), [NPG, NPG],
                                         [1, NPG]])
                        with nc.allow_low_precision(
                                reason="fp32r j-sum feeds fp32r matmul"):
                            nc.vector.tensor_reduce(
                                out=agg[:, c * NPG:(c + 1) * NPG], in_=p3,
                                axis=AX.X, op=ALU.add)
                    ps1 = psum.tile([128, GN], F32, tag="ps")
                    nc.tensor.matmul(ps1, W(f"W2a_{li}"), agg,
                                     start=True, stop=True)
                    # stable softplus: relu(x) + ln(1 + exp(-|x|)); node-path
                    # inputs reach +-250, so the naive exp form overflows
                    a2 = chB.tile([128, GN], F32, tag="exp")
                    nc.scalar.activation(out=a2, in_=ps1, func=AF.Abs,
                                         bias=Bv(f"b2a_{li}"))
                    e2 = chB3.tile([128, GN], F32, tag="xhc")
                    nc.scalar.activation(out=e2, in_=a2, func=AF.Exp,
                                         scale=-1.0)
                    u2 = chB3.tile([128, GN], F32, tag="p")
                    nc.scalar.activation(out=u2, in_=e2, func=AF.Ln, bias=1.0)
                    r2n = chB.tile([128, GN], F32, tag="exp")
                    nc.scalar.activation(out=r2n, in_=ps1, func=AF.Relu,
                                         bias=Bv(f"b2a_{li}"))
                    s2 = chB.tile([128, GN], F32R, tag="x1s")
                    nc.vector.tensor_tensor(out=s2, in0=u2, in1=r2n,
                                            op=ALU.add)
                    ps2 = psum.tile([128, GN], F32, tag="ps")
                    nc.tensor.matmul(ps2, W(f"W3a_{li}"), s2,
                                     start=True, stop=True)
                    nc.vector.scalar_tensor_tensor(
                        out=h, in0=ps2, scalar=Bv(f"b3ap_{li}"), in1=h,
                        op0=ALU.add, op1=ALU.add)
                    if li < NCONV_L:
                        gin_layer(li)

                cond_bc = ap3(cond_sb, 0, [[1, G], [0, NPG]])
                nc.vector.tensor_tensor(out=hg, in0=h, in1=cond_bc,
                                        op=ALU.add)

                nc.vector.tensor_tensor(out=hlc, in0=hl, in1=cond_bc,
                                        op=ALU.add)

            # ---- phase C: pair heads ----
            with tc.tile_pool(name="chC", bufs=2) as chC:

                def head_mlp(tag, prod, attr_t, acol0, out_dram_ap):
                    ps1 = psum.tile([128, CHUNK], F32, tag="ps")
                    mm512(ps1, [W(f"H{tag}_W1a"), W(f"H{tag}_W1b")],
                          [(prod, 0), (attr_t, acol0)], CHUNK)
                    r1 = chC.tile([128, CHUNK], F32R, tag="hr1")
                    nc.scalar.activation(out=r1, in_=ps1, func=AF.Relu,
                                         bias=Bv(f"H{tag}_b1"))
                    ps2 = psum.tile([64, CHUNK], F32, tag="ps")
                    mm512(ps2, [W(f"H{tag}_W2")], [(r1, 0)], CHUNK)
                    r2 = chC.tile([64, CHUNK], F32R, tag="hr2")
                    nc.scalar.activation(out=r2, in_=ps2, func=AF.Relu,
                                         bias=Bv(f"H{tag}_b2")[0:64, :])
                    ps3 = psc.tile([1, CHUNK], F32, tag="psc")
                    mm512(ps3, [W(f"H{tag}_w3")[0:64, :]], [(r2, 0)], CHUNK)
                    oc = chC.tile([1, CHUNK], F32, tag="oc")
                    nc.vector.tensor_copy(oc, ps3)
                    nc.sync.dma_start(out=out_dram_ap, in_=oc)

                def prod_g(c):
                    prod = chC.tile([128, CHUNK], F32R, tag="prod")
                    ia = ap3(hg, c * NPG, [[1, NPG], [0, NPG]])
                    ib = ap3(hg, c * NPG, [[0, NPG], [1, NPG]])
                    nc.gpsimd.tensor_tensor(out=prod, in0=ia, in1=ib,
                                            op=ALU.mult)
                    return prod

                pendp = prod_g(0)
                for c in range(NCH):   # global head, chunk c == graph c
                    prod = pendp
                    if c + 1 < NCH:
                        pendp = prod_g(c + 1)
                    head_mlp("g", prod, attr_g, c * CHUNK,
                             invg_out[:, bass.ts(c, CHUNK)])

                for c in range(NP_B // CHUNK):   # local head: 2 diagonals/chunk
                    prod = chC.tile([128, CHUNK], F32R, tag="prod")
                    nc.vector.tensor_copy(prod, ap3(zeros, 0, [[0, CHUNK]]))
                    for di in (2 * c, 2 * c + 1):
                        d, i0, cnt = binfo[di]
                        col0 = di * GN - c * CHUNK
                        ia = ap3(hlc, i0, [[NPG, G], [1, cnt]])
                        ib = ap3(hlc, i0 + d, [[NPG, G], [1, cnt]])
                        out_ap = ap3(prod, col0 + i0, [[NPG, G], [1, cnt]])
                        nc.vector.tensor_tensor(out=out_ap, in0=ia, in1=ib,
                                                op=ALU.mult)
                    head_mlp("l", prod, attr_l, c * CHUNK,
                             invl_out[:, bass.ts(c, CHUNK)])

    nc.compile()
    return nc


# ---------------- host orchestration ----------------
def _prep(inputs):
    import ml_dtypes

    pos = _np(inputs["pos"])
    spectrum = inputs["spectrum"]
    params = inputs["params"]
    atom_type = np.asarray(inputs["atom_type"], np.int64)
    time_step = np.asarray(inputs["time_step"], np.int64)
    bond_adj = np.asarray(inputs["bond_adj"], np.int32)

    edge_type, dist, valid, local, C = _host_masks(pos, bond_adj)
    wp, bp, (w1g, w1l), (embg, embl) = _prep_params(params)
    cond = _host_cond(spectrum, params, time_step)

    schnet_emb = _np(params["schnet_emb"])
    gin_emb = _np(params["gin_emb"])
    z = atom_type.reshape(B, NPG)
    h0 = schnet_emb[z]
    hl0 = gin_emb[z]

    binfo = _band_indices()
    wmat = wp.matrix()
    bmat = bp.matrix()
    w1row = np.concatenate([w1g.reshape(1, H), w1l.reshape(1, H)], 1)
    in_maps = []
    for core in range(NCORES):
        gs = slice(core * G, (core + 1) * G)
        d_c = dist[gs]
        et_c = edge_type[gs]
        C_c = C[gs].astype(np.float32)

        el_band = np.zeros((ND, G, NPG), np.float32)
        ebg_t = np.zeros((ND, G, NPG, H), np.float32)
        ebl_t = np.zeros((ND, G, NPG, H), np.float32)
        for di, (d, i0, cnt) in enumerate(binfo):
            ii = np.arange(i0, i0 + cnt)
            jj = ii + d
            el_band[di][:, ii] = d_c[:, ii, jj]
            et_d = et_c[:, ii, jj]
            ebg_t[di][:, ii] = embg[et_d]
            ebl_t[di][:, ii] = embl[et_d]

        m = {
            "el_row": np.ascontiguousarray(d_c.reshape(1, NP_D)),
            "el_band": np.ascontiguousarray(el_band.reshape(1, NP_B)),
            "crow": C_c.reshape(1, NP_D).astype(ml_dtypes.bfloat16),
            "ebg": np.ascontiguousarray(ebg_t.reshape(NP_B, H).T),
            "ebl": np.ascontiguousarray(ebl_t.reshape(NP_B, H).T),
            "h0": np.ascontiguousarray(h0[gs].reshape(GN, H).T),
            "hl0": np.ascontiguousarray(hl0[gs].reshape(GN, H).T),
            "cond": np.ascontiguousarray(cond[gs].T),
            "wconst": wmat,
            "bconst": bmat,
            "w1rows": w1row,
        }
        in_maps.append(m)

    hostdata = dict(edge_type=edge_type, dist=dist, valid=valid, local=local,
                    params=params)
    return in_maps, hostdata


def _postprocess(results, hostdata):
    params = hostdata["params"]
    b3g = float(np.asarray(params["mlp_g"][2][1]).reshape(-1)[0])
    b3l = float(np.asarray(params["mlp_l"][2][1]).reshape(-1)[0])
    vf = hostdata["valid"].astype(np.float32)
    lm = hostdata["local"].astype(np.float32)

    inv_g = np.zeros((B, NPG, NPG), np.float32)
    inv_l = np.zeros((B, NPG, NPG), np.float32)
    binfo = _band_indices()
    garr = np.arange(G)
    for core, r in enumerate(results):
        g0 = core * G
        inv_g[g0:g0 + G] = r["invg"].reshape(G, NPG, NPG) + b3g
        band = r["invl"].reshape(ND, G, NPG) + b3l
        for di, (d, i0, cnt) in enumerate(binfo):
            ii = np.arange(i0, i0 + cnt)
            inv_l[(g0 + garr)[:, None], ii[None, :], (ii + d)[None, :]] = \
                band[di][:, ii]
    inv_g = np.where(vf > 0, inv_g, 0.0).astype(np.float32) * vf
    inv_l = np.where(lm > 0, inv_l, 0.0).astype(np.float32) * lm
    return inv_g[..., None], inv_l[..., None]


def _numpy_fallback(inputs, edge_type, dist, valid, local, C):
    """Reference-faithful numpy path, used only if bond_adj is not the
    chain topology the banded device layout assumes."""
    params = inputs["params"]

    def lin(p):
        return _np(p[0]), _np(p[1])

    def sp(x):
        return np.logaddexp(0.0, x).astype(np.float32)

    relu = lambda x: np.maximum(x, 0)
    el = dist[..., None]

    def edge_attr(ep):
        (W1, b1), (W2, b2) = ep["mlp"]
        x1 = relu(el * _np(W1)[None, None, None, 0] + _np(b1))
        return (x1 @ _np(W2) + _np(b2)) * _np(ep["bond_emb"])[edge_type]

    attr_g = edge_attr(params["eenc_g"])
    attr_l = edge_attr(params["eenc_l"])
    cond = _host_cond(inputs["spectrum"], params, inputs["time_step"])
    z = np.asarray(inputs["atom_type"], np.int64).reshape(B, NPG)
    h = _np(params["schnet_emb"])[z]
    Cf = C.astype(np.float32)
    LG2 = np.float32(LOG2)
    for lay in params["schnet"]:
        F1, c1 = lin(lay["filt"][0])
        F2, c2 = lin(lay["filt"][1])
        Wf = ((sp(attr_g @ F1 + c1) - LG2) @ F2 + c2) * Cf[..., None]
        x1 = h @ _np(lay["lin1"])
        agg = np.einsum("bjf,bijf->bif", x1, Wf)
        W2a, b2a = lin(lay["lin2"])
        W3a, b3a = lin(lay["lin"])
        h = h + (sp(agg @ W2a + b2a) - LG2) @ W3a + b3a
    hg = h + cond[:, None, :]
    lm = local.astype(np.float32)
    hl = _np(params["gin_emb"])[z]
    for i, (l1, l2) in enumerate(params["gin"]):
        msg = relu(hl[:, None, :, :] + attr_l) * lm[..., None]
        G1, g1b = lin(l1)
        G2, g2b = lin(l2)
        out = relu((hl + msg.sum(axis=2)) @ G1 + g1b) @ G2 + g2b
        if i < NCONV_L - 1:
            out = relu(out)
        hl = out + hl
    hl = hl + cond[:, None, :]

    def head(hx, attr, mp, mask):
        (W1, b1), (W2, b2), (w3, b3) = [lin(p) for p in mp]
        hp = np.concatenate([hx[:, :, None, :] * hx[:, None, :, :], attr], -1)
        return (relu(relu(hp @ W1 + b1) @ W2 + b2) @ w3 + b3) * mask

    vf = valid.astype(np.float32)[..., None]
    inv_g = head(hg, attr_g, params["mlp_g"], vf)
    inv_l = head(hl, attr_l, params["mlp_l"], lm[..., None])
    return inv_g.astype(np.float32), inv_l.astype(np.float32)


def _is_chain(local):
    band = np.zeros((NPG, NPG), bool)
    for d in (1, 2, 3):
        band |= np.eye(NPG, k=d, dtype=bool) | np.eye(NPG, k=-d, dtype=bool)
    return np.array_equal(local, np.broadcast_to(band, local.shape))


def kernel(pos, spectrum, params, atom_type, time_step, bond_adj):
    in_maps, hostdata = _prep(dict(pos=pos, spectrum=spectrum, params=params,
                                   atom_type=atom_type, time_step=time_step,
                                   bond_adj=bond_adj))
    if not _is_chain(hostdata["local"]):
        inputs = dict(pos=pos, spectrum=spectrum, params=params,
                      atom_type=atom_type, time_step=time_step,
                      bond_adj=bond_adj)
        _, dist, valid, local = (hostdata["edge_type"], hostdata["dist"],
                                 hostdata["valid"], hostdata["local"])
        inv_g, inv_l = _numpy_fallback(inputs, hostdata["edge_type"], dist,
                                       valid, local,
                                       _host_masks(pos, bond_adj)[4])
        el = hostdata["dist"][..., None]
        return (inv_g, inv_l, hostdata["edge_type"], el, hostdata["valid"],
                hostdata["local"])
    if "nc" not in _CACHE:
        wp, bp, _, _ = _prep_params(params)
        _CACHE["nc"] = build_program(wp.slices(), bp.slices(),
                                     wp.matrix().shape[1],
                                     bp.matrix().shape[1])
    res = run_bass_kernel_spmd(_CACHE["nc"], in_maps,
                               core_ids=list(range(NCORES)))
    inv_g, inv_l = _postprocess(res.results, hostdata)
    el = hostdata["dist"][..., None]
    return (inv_g, inv_l, hostdata["edge_type"], el, hostdata["valid"],
            hostdata["local"])
